# revision 1
# baseline (speedup 1.0000x reference)
"""Single-head attention kernel for Trainium2 (8 NeuronCores, SPMD).

Problem: x[4,4096,1024] f32, padding_mask[4,1,4096] i32, Wk/Wq/Wv[64,1024] f32.
  k/q/v = x @ W.T ; wei = softmax(mask(q k^T / 8)) ; out = wei @ v  -> [4,4096,64]

Sharding: core c handles (batch b = c//2, query half = c%2).  Each core gets the
full x[b] (rotated so its 2048 local queries are always rows 0:2048 -- attention
is permutation-invariant over keys, so rotating keys + key-mask together is
exact), computes k/v for all 4096 keys and q for its half, and returns
out[2048, 64].

Device algorithm (per core):
  Phase 1: transpose x via PE transpose (contraction needs C on partitions),
    then project kT/qT/vT = W.T^T @ xT in [H, T] layout; vT is re-transposed
    into v[keys, 64] and extended with a ones column (col 64) used to compute
    softmax denominators via the second matmul.
  Phase 2: per 512-query block: sT[keys,queries] = kT_chunk.T @ qT (PSUM),
    exp via ScalarE with per-partition bias = -1e5*(1-key_mask) (masked keys
    underflow to exactly 0; no row-max subtraction needed since scores are
    O(5)), then oT[65, 512] += v_ext.T @ exp accumulated over key chunks.
    Epilogue: transpose oT back, scale rows by query_mask/denominator.

All large matmuls use float32r operands (TF32-like, 1 cycle/row vs fp32's 4;
measured rel err ~1.5e-4, far inside the f32 envelope for this softmax).
Producers round explicitly (DVE/ACT writes with f32r output dtype).
"""

import sys

if "/opt/trn_rl_repo" not in sys.path:
    sys.path.insert(0, "/opt/trn_rl_repo")

import numpy as np

import concourse.bass as bass
import concourse.mybir as mybir
import concourse.tile as tile
from concourse import bacc
from concourse.bass_utils import run_bass_kernel_spmd

F32 = mybir.dt.float32
F32R = mybir.dt.float32r
T = 4096          # sequence length (keys)
C = 1024          # embedding dim
H = 64            # head size
TBS = 512         # t-block size for phase 1
NTB = T // TBS    # 8 t-blocks
NCC = C // 128    # 8 c-chunks
QL = 2048         # local queries per core
NQB = QL // 512   # 4 query blocks
NKC = T // 128    # 32 key chunks
NEG = -1.0e5      # masked-key bias: exp(s/8 + NEG) underflows to 0.0


def build_nc(reps=1):
    nc = bacc.Bacc("TRN2", target_bir_lowering=False, debug=False, num_devices=8)

    x_d = nc.dram_tensor("x", [T, C], F32, kind="ExternalInput")
    wkt_d = nc.dram_tensor("wkt", [128, NCC, H], F32, kind="ExternalInput")
    wqt_d = nc.dram_tensor("wqt", [128, NCC, H], F32, kind="ExternalInput")
    wvt_d = nc.dram_tensor("wvt", [128, NCC, H], F32, kind="ExternalInput")
    ident_d = nc.dram_tensor("ident", [128, 128], F32, kind="ExternalInput")
    nbias_d = nc.dram_tensor("nbias", [128, NKC], F32, kind="ExternalInput")
    maskq_d = nc.dram_tensor("maskq", [128, QL // 128], F32, kind="ExternalInput")
    out_d = nc.dram_tensor("out", [QL, H], F32, kind="ExternalOutput")

    with tile.TileContext(nc) as tc:
        with (
            tc.tile_pool(name="const", bufs=1) as const,
            tc.tile_pool(name="persist", bufs=1) as persist,
            tc.tile_pool(name="xin", bufs=2) as xin,
            tc.tile_pool(name="xt", bufs=3) as xtp,
            tc.tile_pool(name="vt", bufs=2) as vtp,
            tc.tile_pool(name="expp", bufs=4) as expp,
            tc.tile_pool(name="osb", bufs=2) as osb,
            tc.tile_pool(name="small", bufs=4) as small,
            tc.tile_pool(name="psA", bufs=2, space=bass.MemorySpace.PSUM) as psA,
            tc.tile_pool(name="psB", bufs=1, space=bass.MemorySpace.PSUM) as psB,
            tc.tile_pool(name="psC", bufs=2, space=bass.MemorySpace.PSUM) as psC,
            tc.tile_pool(name="psD", bufs=1, space=bass.MemorySpace.PSUM) as psD,
        ):
            pools = (const, persist, xin, xtp, vtp, expp, osb, small,
                     psA, psB, psC, psD)
            drams = (x_d, wkt_d, wqt_d, wvt_d, ident_d, nbias_d, maskq_d, out_d)

            if reps == 1:
                _emit(nc, pools, drams)
            else:
                with tc.For_i(0, reps):
                    _emit(nc, pools, drams)

    nc.compile()
    return nc


def _emit(nc, pools, drams):
    (const, persist, xin, xtp, vtp, expp, osb, small,
     psA, psB, psC, psD) = pools
    (x_d, wkt_d, wqt_d, wvt_d, ident_d, nbias_d, maskq_d, out_d) = drams

    # ---- constants ----
    wstage = const.tile([128, 3, NCC, H], F32)
    nc.sync.dma_start(out=wstage[:, 0], in_=wkt_d.ap())
    nc.sync.dma_start(out=wstage[:, 1], in_=wqt_d.ap())
    nc.sync.dma_start(out=wstage[:, 2], in_=wvt_d.ap())
    w_sb = const.tile([128, 3, NCC, H], F32R)   # rounded for f32r matmuls
    nc.vector.tensor_copy(w_sb, wstage)
    wkt_sb, wqt_sb, wvt_sb = w_sb[:, 0], w_sb[:, 1], w_sb[:, 2]
    ident_sb = const.tile([128, 128], F32)
    nbias_sb = const.tile([128, NKC], F32)
    maskq_sb = const.tile([128, QL // 128], F32)
    nc.sync.dma_start(out=ident_sb, in_=ident_d.ap())
    nc.sync.dma_start(out=nbias_sb, in_=nbias_d.ap())
    nc.sync.dma_start(out=maskq_sb, in_=maskq_d.ap())

    # ---- persistent intermediates ----
    kT_sb = persist.tile([H, T], F32R)           # k^T  [64, 4096]
    qT_sb = persist.tile([H, QL], F32R)          # q^T  [64, 2048]
    v_sb = persist.tile([128, NKC, H + 1], F32R)  # v_ext [keys, 65]
    out_acc = persist.tile([128, QL // 128, H], F32)
    ones_sb = const.tile([128, NKC], F32)
    nc.vector.memset(ones_sb, 1.0)
    nc.vector.tensor_copy(v_sb[:, :, H], ones_sb)  # ones column (rounds to f32r)

    # ================= Phase 1: transpose + projections ============
    for tb in range(NTB):
        x_tile = xin.tile([128, TBS // 128, C], F32)
        nc.sync.dma_start(
            out=x_tile,
            in_=x_d.ap()[tb * TBS : (tb + 1) * TBS, :].rearrange(
                "(s p) c -> p s c", p=128
            ),
        )
        kqv_ps = psB.tile([H, 3, TBS], F32)
        for cc in range(NCC):
            # transpose x[tb, cc]: 4x [128t,128c] -> [128c, 512t]
            tp_ps = psA.tile([128, TBS], F32, tag="pa")
            for s in range(TBS // 128):
                nc.tensor.transpose(
                    tp_ps[:, s * 128 : (s + 1) * 128],
                    x_tile[:, s, cc * 128 : (cc + 1) * 128],
                    ident_sb,
                )
            xT_sb = xtp.tile([128, TBS], F32R)
            # alternate ACT/DVE for PSUM->SBUF copies to split the load
            if cc % 2 == 0:
                nc.scalar.copy(xT_sb, tp_ps)
            else:
                nc.vector.tensor_copy(xT_sb, tp_ps)
            first, last = cc == 0, cc == NCC - 1
            nc.tensor.matmul(
                kqv_ps[:, 0, :], wkt_sb[:, cc, :], xT_sb, start=first, stop=last
            )
            nc.tensor.matmul(
                kqv_ps[:, 1, :], wvt_sb[:, cc, :], xT_sb, start=first, stop=last
            )
            if tb < NQB:  # local queries are always rows 0:2048
                nc.tensor.matmul(
                    kqv_ps[:, 2, :], wqt_sb[:, cc, :], xT_sb, start=first, stop=last
                )
        nc.vector.tensor_copy(kT_sb[:, tb * TBS : (tb + 1) * TBS], kqv_ps[:, 0, :])
        if tb < NQB:
            nc.vector.tensor_copy(qT_sb[:, tb * TBS : (tb + 1) * TBS], kqv_ps[:, 2, :])
        # vT -> v (re-transpose to [keys, 64] layout)
        vT_sb = vtp.tile([H, TBS], F32)
        nc.scalar.copy(vT_sb, kqv_ps[:, 1, :])
        vtp_ps = psC.tile([128, TBS // 128, H], F32, tag="small")
        for s in range(TBS // 128):
            nc.tensor.transpose(
                vtp_ps[:, s, :],
                vT_sb[:, s * 128 : (s + 1) * 128],
                ident_sb[:H, :H],
            )
        nc.vector.tensor_copy(
            v_sb[:, tb * (TBS // 128) : (tb + 1) * (TBS // 128), 0:H], vtp_ps
        )

    # ================= Phase 2: attention =========================
    for qb in range(NQB):
        oT_ps = psD.tile([H + 1, 512], F32)
        for kc in range(NKC):
            sT_ps = psA.tile([128, 512], F32, tag="pa")
            nc.tensor.matmul(
                sT_ps,
                kT_sb[:, kc * 128 : (kc + 1) * 128],
                qT_sb[:, qb * 512 : (qb + 1) * 512],
                start=True,
                stop=True,
            )
            exp_sb = expp.tile([128, 512], F32R)
            nc.scalar.activation(
                exp_sb,
                sT_ps,
                mybir.ActivationFunctionType.Exp,
                bias=nbias_sb[:, kc : kc + 1],
                scale=0.125,
            )
            nc.tensor.matmul(
                oT_ps,
                v_sb[:, kc, :],
                exp_sb,
                start=(kc == 0),
                stop=(kc == NKC - 1),
            )
        oT_sb = osb.tile([H + 1, 512], F32)
        nc.vector.tensor_copy(oT_sb, oT_ps)
        for qs in range(4):
            qt = qb * 4 + qs
            ot_ps = psC.tile([128, H + 1], F32, tag="small")
            nc.tensor.transpose(
                ot_ps,
                oT_sb[:, qs * 128 : (qs + 1) * 128],
                ident_sb[: H + 1, : H + 1],
            )
            recip_sb = small.tile([128, 1], F32)
            nc.vector.reciprocal(recip_sb, ot_ps[:, H : H + 1])
            nc.vector.tensor_scalar(
                out=out_acc[:, qt, :],
                in0=ot_ps[:, 0:H],
                scalar1=recip_sb,
                scalar2=maskq_sb[:, qt : qt + 1],
                op0=mybir.AluOpType.mult,
                op1=mybir.AluOpType.mult,
            )
    nc.sync.dma_start(
        out=out_d.ap().rearrange("(n p) h -> p n h", p=128), in_=out_acc
    )


_NC_CACHE = None


def _get_nc():
    global _NC_CACHE
    if _NC_CACHE is None:
        _NC_CACHE = build_nc()
    return _NC_CACHE


def build_nc_reps(reps):
    return build_nc(reps=reps)


def _prep_core_inputs(x, padding_mask, wkt, wqt, wvt, ident, core):
    b, half = core // 2, core % 2
    q0 = half * QL
    xb = x[b]
    m = padding_mask[b, 0].astype(np.float32)
    if half:  # rotate keys so local queries are rows 0:2048 (exact: permutation
        # of keys with identically-permuted key mask leaves attention unchanged)
        xb = np.concatenate([xb[q0:], xb[:q0]], axis=0)
        m = np.concatenate([m[q0:], m[:q0]], axis=0)
    nbias = np.ascontiguousarray((NEG * (1.0 - m)).reshape(NKC, 128).T)
    maskq = np.ascontiguousarray(m[:QL].reshape(QL // 128, 128).T)
    return {
        "x": np.ascontiguousarray(xb),
        "wkt": wkt,
        "wqt": wqt,
        "wvt": wvt,
        "ident": ident,
        "nbias": nbias,
        "maskq": maskq,
    }


def make_in_maps(x, padding_mask, Wk, Wq, Wv):
    def wt(w):  # [64,1024] -> [128, 8, 64]: wt[p, cc, h] = w[h, cc*128+p]
        return np.ascontiguousarray(w.T.reshape(NCC, 128, H).transpose(1, 0, 2))

    wkt, wqt, wvt = wt(np.asarray(Wk)), wt(np.asarray(Wq)), wt(np.asarray(Wv))
    ident = np.eye(128, dtype=np.float32)
    x = np.asarray(x)
    padding_mask = np.asarray(padding_mask)
    return [
        _prep_core_inputs(x, padding_mask, wkt, wqt, wvt, ident, c) for c in range(8)
    ]


def kernel(x, padding_mask, Wk, Wq, Wv):
    nc = _get_nc()
    in_maps = make_in_maps(x, padding_mask, Wk, Wq, Wv)
    res = run_bass_kernel_spmd(nc, in_maps, core_ids=list(range(8)), trace=False)
    B = x.shape[0]
    out = np.empty((B, T, H), dtype=np.float32)
    for c in range(8):
        b, half = c // 2, c % 2
        out[b, half * QL : (half + 1) * QL, :] = res.results[c]["out"]
    return out



# revision 11
# speedup vs baseline: 3.7395x; 3.7395x over previous
"""Single-head attention kernel for Trainium2 (8 NeuronCores, SPMD).

Problem: x[4,4096,1024] f32, padding_mask[4,1,4096] i32, Wk/Wq/Wv[64,1024] f32.
  k/q/v = x @ W.T ; wei = softmax(mask(q k^T / 8)) ; out = wei @ v  -> [4,4096,64]

Structural wins over the naive mapping:
  * Compaction: masked key columns contribute 0 weight and masked query rows
    output exactly 0, and the key mask equals the query mask.  The host gathers
    each batch's ~2048 unmasked rows, pads to NU (multiple of 256), and the
    device computes attention only over the compacted set.  Pad rows are zero,
    so their v-contribution is 0; the softmax denominator is computed via an
    extra "ones" column in v that the host zeroes for pad keys -- no masking
    bias is needed anywhere on the device.
  * Host pre-transposes x to xT[bf16] so the device does no PE transposes of x
    and HBM traffic is halved (bf16).
  * All matmul operands are bf16 (1 cycle/row, FWL weight loads); PSUM stays
    f32.
  * Sharding: 2 cores per batch; core half h computes compacted queries
    [r, r+qL) where r = nU-qL for h=1 (keys rotated by r, which is an exact
    permutation invariance), so no computed query row is wasted on padding.
  * Projections pack [k|v] into one 128-col stationary; odd t-blocks use
    [v|k] so kT lands at PSUM partitions 64:128 for them.  That gives score
    chunks at both partition bases, enabling ROW-TILED score matmuls: two
    K=64 matmuls run concurrently in the PE array (rows 0:63 and 64:127).
    q uses a duplicated [wq|wq] stationary so qT is born replicated on both
    partition halves.
"""

import sys

if "/opt/trn_rl_repo" not in sys.path:
    sys.path.insert(0, "/opt/trn_rl_repo")

import numpy as np
import ml_dtypes

import concourse.bass as bass
import concourse.mybir as mybir
import concourse.tile as tile
from concourse import bacc
from concourse.bass_utils import run_bass_kernel_spmd

F32 = mybir.dt.float32
BF16 = mybir.dt.bfloat16
BF_NP = ml_dtypes.bfloat16
B, T, C, H = 4, 4096, 1024, 64
NCC = C // 128  # 8 c-chunks


def _blocks(total, step):
    out, t0 = [], 0
    while t0 < total:
        out.append((t0, min(step, total - t0)))
        t0 += step
    return out


def build_nc(NU, parity=True, force_singles=False):
    qL = NU // 2
    NKC = NU // 128
    nc = bacc.Bacc("TRN2", target_bir_lowering=False, debug=False, num_devices=8)

    xT_d = nc.dram_tensor("xt", [C, NU], BF16, kind="ExternalInput")
    wkv_d = nc.dram_tensor("wkv", [128, NCC, 128], BF16, kind="ExternalInput")
    wvk_d = nc.dram_tensor("wvk", [128, NCC, 128], BF16, kind="ExternalInput")
    wqd_d = nc.dram_tensor("wqd", [128, NCC, 128], BF16, kind="ExternalInput")
    identd_d = nc.dram_tensor("identd", [128, 64], F32, kind="ExternalInput")
    identf_d = nc.dram_tensor("identf", [128, 128], F32, kind="ExternalInput")
    onesk_d = nc.dram_tensor("onesk", [128, NKC], F32, kind="ExternalInput")
    out_d = nc.dram_tensor("out", [qL, H], F32, kind="ExternalOutput")

    tb_blocks = _blocks(NU, 512)
    qb_blocks = _blocks(qL, 512)

    # key-chunk -> (owning t-block, psum partition base of kT for that block)
    kc_tb = []
    for i, (t0, tbs) in enumerate(tb_blocks):
        for _ in range(tbs // 128):
            kc_tb.append(i)
    base_k = [0 if (kc_tb[kc] % 2 == 0 or not parity) else 64 for kc in range(NKC)]
    evens = [kc for kc in range(NKC) if base_k[kc] == 0]
    odds = [kc for kc in range(NKC) if base_k[kc] == 64]
    npairs = 0 if force_singles else min(len(evens), len(odds))
    pairs = list(zip(evens[:npairs], odds[:npairs]))
    singles = evens[npairs:] + odds[npairs:]

    with tile.TileContext(nc) as tc:
        with (
            tc.tile_pool(name="const", bufs=1) as const,
            tc.tile_pool(name="persist", bufs=1) as persist,
            tc.tile_pool(name="expp", bufs=3) as expp,
            tc.tile_pool(name="osb", bufs=2) as osb,
            tc.tile_pool(name="small", bufs=4) as small,
            tc.tile_pool(name="psP", bufs=2, space=bass.MemorySpace.PSUM) as psP,
            tc.tile_pool(name="psS", bufs=2, space=bass.MemorySpace.PSUM) as psS,
            tc.tile_pool(name="psO", bufs=2, space=bass.MemorySpace.PSUM) as psO,
        ):
            # ---- constants ----
            wkv_sb = const.tile([128, NCC, 128], BF16)
            wvk_sb = const.tile([128, NCC, 128], BF16)
            wqd_sb = const.tile([128, NCC, 128], BF16)
            identd_sb = const.tile([128, 64], F32)
            identf_sb = const.tile([128, 128], F32)
            onesk_sb = const.tile([128, NKC], F32)
            nc.sync.dma_start(out=wkv_sb, in_=wkv_d.ap())
            nc.sync.dma_start(out=wvk_sb, in_=wvk_d.ap())
            nc.sync.dma_start(out=wqd_sb, in_=wqd_d.ap())
            nc.sync.dma_start(out=identd_sb, in_=identd_d.ap())
            nc.sync.dma_start(out=identf_sb, in_=identf_d.ap())
            nc.sync.dma_start(out=onesk_sb, in_=onesk_d.ap())

            # ---- persistent intermediates ----
            xT_sb = persist.tile([128, NCC, NU], BF16)
            kT_rep = persist.tile([128, NU], BF16)
            qT_rep = persist.tile([128, qL], BF16)
            vT_sb = persist.tile([128, NU], F32)
            v_sb = persist.tile([128, NKC, H + 1], BF16)
            out_acc = persist.tile([128, qL // 128, H], F32)
            nc.vector.tensor_copy(v_sb[:, :, H], onesk_sb)

            # x chunks, in consumption order
            for t0, tbs in tb_blocks:
                for cc in range(NCC):
                    nc.sync.dma_start(
                        out=xT_sb[:, cc, t0 : t0 + tbs],
                        in_=xT_d.ap()[cc * 128 : (cc + 1) * 128, t0 : t0 + tbs],
                    )

            # ============ Phase 1: projections =============
            for i, (t0, tbs) in enumerate(tb_blocks):
                even = (i % 2 == 0) or not parity
                w_sb = wkv_sb if even else wvk_sb
                bk, bv = (0, 64) if even else (64, 0)
                kv_ps = psP.tile([128, 512], F32, tag="p", name="kv_ps")
                for cc in range(NCC):
                    nc.tensor.matmul(
                        kv_ps[:, :tbs],
                        w_sb[:, cc, :],
                        xT_sb[:, cc, t0 : t0 + tbs],
                        start=(cc == 0),
                        stop=(cc == NCC - 1),
                    )
                # kT to its parity base; vT to the other
                nc.scalar.copy(
                    kT_rep[bk : bk + 64, t0 : t0 + tbs], kv_ps[bk : bk + 64, :tbs]
                )
                nc.vector.tensor_copy(
                    vT_sb[bv : bv + 64, t0 : t0 + tbs], kv_ps[bv : bv + 64, :tbs]
                )
                if t0 < qL:  # q projection (duplicated on both halves)
                    qbs = min(tbs, qL - t0)
                    q_ps = psP.tile([128, 512], F32, tag="p", name="q_ps")
                    for cc in range(NCC):
                        nc.tensor.matmul(
                            q_ps[:, :qbs],
                            wqd_sb[:, cc, :],
                            xT_sb[:, cc, t0 : t0 + qbs],
                            start=(cc == 0),
                            stop=(cc == NCC - 1),
                        )
                    nc.vector.tensor_copy(
                        qT_rep[:, t0 : t0 + qbs], q_ps[:, :qbs]
                    )
                # v re-transpose: [64, 128] chunks -> [128, 64]
                for j in range(tbs // 128):
                    kc = t0 // 128 + j
                    vt_ps = psP.tile([128, 512], F32, tag="p", name="vt_ps")
                    nc.tensor.transpose(
                        vt_ps[:, 0:H],
                        vT_sb[bv : bv + 64, kc * 128 : (kc + 1) * 128],
                        identd_sb[bv : bv + 64, :],
                    )
                    nc.vector.tensor_copy(v_sb[:, kc, 0:H], vt_ps[:, 0:H])

            # ============ Phase 2: attention =============
            chunk_groups = [list(p) for p in pairs] + [[kc] for kc in singles]
            ng = len(chunk_groups)
            for q0, qbs in qb_blocks:
                oT_ps = psO.tile([H + 1, 512], F32, name="oT_ps")
                groups = []  # exp tiles, one per chunk group, in AV order
                offs = (0, 512)  # B half on its own PSUM bank
                for g, kcs in enumerate(chunk_groups):
                    sT_ps = psS.tile([128, 1024], F32, tag="s", name="sT_ps")
                    for idx, kc in enumerate(kcs):
                        bkc = base_k[kc]
                        nc.tensor.matmul(
                            sT_ps[:, offs[idx] : offs[idx] + qbs],
                            kT_rep[bkc : bkc + 64, kc * 128 : (kc + 1) * 128],
                            qT_rep[bkc : bkc + 64, q0 : q0 + qbs],
                            start=True,
                            stop=True,
                        )
                    exp_sb = expp.tile([128, 1024], BF16, name="exp_sb")
                    if len(kcs) == 2 and qbs == 512:
                        nc.scalar.activation(
                            exp_sb[:, 0:1024],
                            sT_ps[:, 0:1024],
                            mybir.ActivationFunctionType.Exp,
                            scale=0.125,
                        )
                    else:
                        for idx in range(len(kcs)):
                            nc.scalar.activation(
                                exp_sb[:, offs[idx] : offs[idx] + qbs],
                                sT_ps[:, offs[idx] : offs[idx] + qbs],
                                mybir.ActivationFunctionType.Exp,
                                scale=0.125,
                            )
                    groups.append((exp_sb, kcs))
                    if g > 0:  # stay one group ahead of ACT on the PE
                        _emit_av(nc, oT_ps, v_sb, groups[g - 1], qbs, offs,
                                 first=(g - 1 == 0), last=False)
                _emit_av(nc, oT_ps, v_sb, groups[-1], qbs, offs,
                         first=(ng == 1), last=True)

                # epilogue: transpose back, scale by 1/denominator
                oT_sb = osb.tile([H + 1, 512], F32, name="oT_sb")
                nc.vector.tensor_copy(oT_sb[:, :qbs], oT_ps[:, :qbs])
                for qs in range(qbs // 128):
                    qt = q0 // 128 + qs
                    ot_ps = psS.tile([128, 1024], F32, tag="s", name="ot_ps")
                    nc.tensor.transpose(
                        ot_ps[:, 0 : H + 1],
                        oT_sb[:, qs * 128 : (qs + 1) * 128],
                        identf_sb[: H + 1, : H + 1],
                    )
                    recip_sb = small.tile([128, 1], F32, name="recip_sb")
                    nc.vector.reciprocal(recip_sb, ot_ps[:, H : H + 1])
                    nc.vector.tensor_scalar_mul(
                        out_acc[:, qt, :], ot_ps[:, 0:H], recip_sb
                    )
            nc.sync.dma_start(
                out=out_d.ap().rearrange("(n p) h -> p n h", p=128), in_=out_acc
            )

    nc.compile()
    return nc


def _emit_av(nc, oT_ps, v_sb, group, qbs, offs, first, last):
    exp_sb, kcs = group
    for idx, kc in enumerate(kcs):
        nc.tensor.matmul(
            oT_ps[:, 0:qbs],
            v_sb[:, kc, :],
            exp_sb[:, offs[idx] : offs[idx] + qbs],
            start=(first and idx == 0),
            stop=(last and idx == len(kcs) - 1),
        )


_NC_CACHE = {}


def _get_nc(NU):
    if NU not in _NC_CACHE:
        _NC_CACHE[NU] = build_nc(NU)
    return _NC_CACHE[NU]


def make_in_maps(x, padding_mask, Wk, Wq, Wv):
    x = np.asarray(x)
    padding_mask = np.asarray(padding_mask)
    Wk, Wq, Wv = (np.asarray(w, np.float32) for w in (Wk, Wq, Wv))

    idxs = [np.nonzero(padding_mask[b, 0])[0] for b in range(B)]
    nUs = [len(ix) for ix in idxs]
    NU = max(256, -(-max(nUs) // 256) * 256)
    qL = NU // 2
    NKC = NU // 128

    def wt(w):  # [64,1024] -> [128, NCC, 64]: wt[p, cc, h] = w[h, cc*128+p]
        return w.T.reshape(NCC, 128, H).transpose(1, 0, 2)

    wk, wq, wv = wt(Wk), wt(Wq), wt(Wv)
    wkv = np.concatenate([wk, wv], axis=2).astype(BF_NP)
    wvk = np.concatenate([wv, wk], axis=2).astype(BF_NP)
    wqd = np.concatenate([wq, wq], axis=2).astype(BF_NP)
    identd = np.ascontiguousarray(
        np.vstack([np.eye(64, dtype=np.float32)] * 2)
    )
    identf = np.eye(128, dtype=np.float32)

    in_maps = []
    for b in range(B):
        ix, nU = idxs[b], nUs[b]
        x_u = np.zeros((NU, C), np.float32)
        x_u[:nU] = x[b][ix]
        xT0 = np.ascontiguousarray(x_u.T).astype(BF_NP)
        ones = np.zeros(NU, np.float32)
        ones[:nU] = 1.0
        for h in range(2):
            r = max(nU - qL, 0) if h else 0
            if r:
                xT = np.ascontiguousarray(np.roll(xT0, -r, axis=1))
                ones_r = np.roll(ones, -r)
            else:
                xT, ones_r = xT0, ones
            onesk = np.ascontiguousarray(ones_r.reshape(NKC, 128).T)
            in_maps.append(
                {
                    "xt": xT,
                    "wkv": wkv,
                    "wvk": wvk,
                    "wqd": wqd,
                    "identd": identd,
                    "identf": identf,
                    "onesk": onesk,
                }
            )
    return NU, idxs, nUs, in_maps


def kernel(x, padding_mask, Wk, Wq, Wv):
    NU, idxs, nUs, in_maps = make_in_maps(x, padding_mask, Wk, Wq, Wv)
    qL = NU // 2
    nc = _get_nc(NU)
    res = run_bass_kernel_spmd(nc, in_maps, core_ids=list(range(8)), trace=False)
    out = np.zeros((B, T, H), dtype=np.float32)
    for b in range(B):
        ix, nU = idxs[b], nUs[b]
        res0 = res.results[2 * b]["out"]
        res1 = res.results[2 * b + 1]["out"]
        n0 = min(qL, nU)
        out[b, ix[0:n0]] = res0[0:n0]
        r = max(nU - qL, 0)
        out[b, ix[r:nU]] = res1[0 : nU - r]
    return out
